# revision 10
# baseline (speedup 1.0000x reference)
"""Trainium2 Bass kernel for the BIGA E/I recurrent network (gnn_message_passing).

Architecture recap (hardcoded shapes):
  tokens (16,128) -> x = emb[tokens]+pe -> ext = x @ W_in.T  (host, fp32)
  128 sequential Euler steps over 4 coupled E/I groups (S, A1, A2, G),
  per-step readout logits_t = ge @ W_out.T with V=32000.

Device strategy (8 NeuronCores, SPMD):
  - The recurrence is replicated on every core (B=16 gives no cross-core
    scaling: matmul cost is weight-streaming-bound, and per-step collectives
    have a ~20us floor).  All recurrence matmuls run in fp16 with the batch
    (M=16) as the stationary free dim and 4-way PE column tiling
    (tile_position=(0,32g)) so four weight streams run concurrently.
  - States are kept TRANSPOSED (yT[k,b], fp16) so they can be used directly
    as matmul stationaries; each step the new pre-activations are
    relu(0.1*x)-evacuated by ScalarE, transposed back in batched 128x128 PE
    transposes, and combined y' = 0.9*y + r on VectorE.
  - The V=32000 output projection is sharded over vocab: each core projects
    all T*B=2048 ge states onto its 4000-row slice of W_out with full
    128x128 matmuls, then the host concatenates.
"""

import os
import sys

import numpy as np

try:
    import concourse.bass as bass  # noqa: F401
except ImportError:
    for p in ("/opt/trn_rl_repo", "/root/.axon_site/_ro/trn_rl_repo"):
        if os.path.isdir(p) and p not in sys.path:
            sys.path.insert(0, p)

import concourse.bass as bass
import concourse.mybir as mybir
import concourse.tile as tile
from concourse import bacc
from concourse.bass_utils import run_bass_kernel_spmd

F16 = mybir.dt.float16
F32 = mybir.dt.float32

V, D, NE, NI, B, T_FULL = 32000, 512, 1024, 256, 16, 128
DT = 0.1
NCORES = 8
VSH = V // NCORES          # 4000 vocab rows per core
NCH = 500                  # projection free-dim chunk (4000 = 8*500)
KT_E = NE // 128           # 8 k-tiles per E state
KT_I = NI // 128           # 2 k-tiles per I state

# state name -> (is_E, index)
E_STATES = ["se", "a1e", "a2e", "ge"]
I_STATES = ["si", "a1i", "a2i", "gi"]
I_SLOT = {"si": 0, "a1i": 1, "a2i": 2, "gi": 3}

# Per-destination contraction orders.  Host weight packing MUST match.
#   entry = (src_state, k_tile)
def _ks(s, n, o=0):
    return [(s, k + o) for k in range(n)]

CONTR_E = {
    "S":  _ks("se", 8) + _ks("si", 2),                    # + ext via eyeE
    "A1": _ks("a1e", 8) + _ks("a1i", 2) + _ks("se", 4) + _ks("a2i", 2),
    "A2": _ks("a2e", 8) + _ks("a2i", 2) + _ks("se", 4, 4) + _ks("a1i", 2),
    "G":  _ks("ge", 8) + _ks("gi", 2) + _ks("a1e", 8) + _ks("a2e", 8),
}
CONTR_I = {
    "S":  _ks("se", 8) + _ks("si", 2),
    "A1": _ks("a1e", 8) + _ks("a1i", 2),
    "A2": _ks("a2e", 8) + _ks("a2i", 2),
    "G":  _ks("ge", 8) + _ks("gi", 2),
}
GROUPS = ["S", "A1", "A2", "G"]
E_OF = {"S": "se", "A1": "a1e", "A2": "a2e", "G": "ge"}
I_OF = {"S": "si", "A1": "a1i", "A2": "a2i", "G": "gi"}

# Column-group work assignment (PE 128x32 col tiling).  Each col group g
# owns 3 PSUM banks (slots); slot u of group g accumulates one unit:
#   slot 0 (free [0:512)):    g0: G[0:512)   g1: G[512:1024)  g2: A1[0:512) g3: A1[512:1024)
#   slot 1 (free [512:1024)): g0: S[0:512)   g1: S[512:1024)  g2: A2[0:512) g3: A2[512:1024)
#   slot 2 (free [1024:1280)): I unit of group g's own state (N=256)
# Per-group cycles: g0/g1: 26*512+11*512+10*256 = 21504; g2/g3: 18944.
SLOT_E = {  # (slot, g) -> (dst_group, n0)
    (0, 0): ("G", 0), (0, 1): ("G", 512), (0, 2): ("A1", 0), (0, 3): ("A1", 512),
    (1, 0): ("S", 0), (1, 1): ("S", 512), (1, 2): ("A2", 0), (1, 3): ("A2", 512),
}
PRE_W = 1536  # psum_pre / staging free width (3 banks per partition-group)


def _pack_kxn(blocks):
    """blocks: list of [128, N] fp32 arrays (the k-tile slices of W.T, in
    contraction order).  Returns [128, nk*N] fp16 (k-tile-major columns)."""
    a = np.stack(blocks, axis=0)            # [nk, 128, N]
    a = np.transpose(a, (1, 0, 2))          # [128, nk, N]
    return np.ascontiguousarray(a.reshape(128, -1)).astype(np.float16)


def _kslices(wT, nk):
    """wT: [K, N] with K = nk*128 -> list of nk [128, N] blocks."""
    return [wT[128 * k:128 * (k + 1)] for k in range(nk)]


def host_prepare(inputs):
    """All constant/layout prep in numpy. Returns (common_map, per_core_maps)."""
    f = {k: np.asarray(v, dtype=np.float32) if np.asarray(v).dtype != np.int64
         and np.asarray(v).dtype != np.int32 else np.asarray(v)
         for k, v in inputs.items()}
    tokens = np.asarray(inputs["tokens"]).astype(np.int64)

    # ext = (emb[tokens] + pe) @ W_in.T  -> (T, B, NE) -> [T*B, NE] fp16
    x = f["embedding"][tokens] + f["pos_enc"][None, :T_FULL]      # (B,T,D)
    ext = np.einsum("btd,nd->btn", x.astype(np.float32),
                    f["W_in"].astype(np.float32))                  # (B,T,NE)
    ext = np.transpose(ext, (1, 0, 2)).reshape(T_FULL * B, NE)     # t-major
    ext16 = ext.astype(np.float16)

    wmap = {}
    # E-destination packs (N=1024): order must match CONTR_E
    wmap["w_S"] = _pack_kxn(_kslices(f["S_Wee"].T, 8) + _kslices(-f["S_Wei"].T, 2))
    wmap["w_A1"] = _pack_kxn(
        _kslices(f["A1_Wee"].T, 8) + _kslices(-f["A1_Wei"].T, 2)
        + _kslices(f["C_S_A1"].T[:512], 4) + _kslices(-f["L_A2_A1"].T, 2))
    wmap["w_A2"] = _pack_kxn(
        _kslices(f["A2_Wee"].T, 8) + _kslices(-f["A2_Wei"].T, 2)
        + _kslices(f["C_S_A2"].T[512:], 4) + _kslices(-f["L_A1_A2"].T, 2))
    wmap["w_G"] = _pack_kxn(
        _kslices(f["G_Wee"].T, 8) + _kslices(-f["G_Wei"].T, 2)
        + _kslices(f["C_A1_G"].T, 8) + _kslices(f["C_A2_G"].T, 8))
    # I-destination packs (N=256)
    for gname in GROUPS:
        wmap[f"w_{gname}i"] = _pack_kxn(
            _kslices(f[f"{gname}_Wie"].T, 8) + _kslices(-f[f"{gname}_Wii"].T, 2))

    wmap["eye_e"] = np.eye(128, 16, dtype=np.float16)
    wmap["ident"] = np.eye(128, 128, dtype=np.float16)

    common = dict(ext=ext16, **wmap)

    # per-core W_out vocab shard, packed k-tile-major: [128, 8*4000]
    per_core = []
    for c in range(NCORES):
        wsh = f["W_out"][c * VSH:(c + 1) * VSH]        # [4000, NE]
        wT = wsh.T                                     # [NE, 4000]
        per_core.append({"w_out": _pack_kxn(_kslices(wT, 8))})
    return common, per_core


def build_program(t_steps=T_FULL, reps=1):
    nc = bacc.Bacc("TRN2", target_bir_lowering=False, debug=False,
                   num_devices=NCORES)

    # ---- DRAM I/O ----
    d_ext = nc.dram_tensor("ext", [T_FULL * B, NE], F16, kind="ExternalInput")
    d_w = {}
    for gname in GROUPS:
        d_w[gname] = nc.dram_tensor(f"w_{gname}", [128, len(CONTR_E[gname]) * NE],
                                    F16, kind="ExternalInput")
        d_w[gname + "i"] = nc.dram_tensor(f"w_{gname}i", [128, len(CONTR_I[gname]) * NI],
                                          F16, kind="ExternalInput")
    d_eye = nc.dram_tensor("eye_e", [128, 16], F16, kind="ExternalInput")
    d_ident = nc.dram_tensor("ident", [128, 128], F16, kind="ExternalInput")
    d_wout = nc.dram_tensor("w_out", [128, KT_E * VSH], F16, kind="ExternalInput")
    d_gt = nc.dram_tensor("gt_buf", [T_FULL, 128, 128], F16)
    d_log = nc.dram_tensor("logits", [T_FULL * B, VSH], F32, kind="ExternalOutput")

    from contextlib import ExitStack
    with tile.TileContext(nc) as tc, ExitStack() as _rep_ctx:
        if reps > 1:
            # timing-only variant: repeat the whole kernel body on-device so
            # wall-clock deltas isolate pure device execution time
            _rep_ctx.enter_context(tc.For_i(0, reps, 1))
        # ================= Phase B: recurrence =================
        with (
            tc.tile_pool(name="wpool", bufs=1) as wpool,
            tc.tile_pool(name="state", bufs=1) as spool,
            tc.tile_pool(name="ext", bufs=1) as epool,
            tc.tile_pool(name="stag", bufs=1) as stpool,
            tc.tile_pool(name="pre", bufs=1, space="PSUM") as prepool,
            tc.tile_pool(name="tr", bufs=3, space="PSUM") as trpool,
        ):
            # weights
            w_sb = {}
            for gname in GROUPS:
                w_sb[gname] = wpool.tile([128, len(CONTR_E[gname]) * NE], F16,
                                         name=f"w_{gname}", tag=f"w{gname}")
                nc.sync.dma_start(w_sb[gname][:], d_w[gname][:])
                w_sb[gname + "i"] = wpool.tile([128, len(CONTR_I[gname]) * NI],
                                               F16, name=f"w_{gname}i", tag=f"w{gname}i")
                nc.sync.dma_start(w_sb[gname + "i"][:], d_w[gname + "i"][:])
            eye_e = wpool.tile([128, 16], F16, name="eye_e_sb", tag="eye")
            nc.sync.dma_start(eye_e[:], d_eye[:])
            ident = wpool.tile([128, 128], F16, name="ident_sb", tag="ident")
            nc.sync.dma_start(ident[:], d_ident[:])

            # transposed states, double-buffered by step parity.
            # E states live in ONE [128, 512] tile per parity, state order
            # EORDER (chosen so each transpose window's 4 psum col-blocks map
            # to one strided AP).  I states: [128, 128], offset (4c+s)*16.
            EORDER = ["ge", "a1e", "se", "a2e"]
            yTE = [spool.tile([128, 512], F16, name=f"yTE_{p}", tag=f"yTE_{p}")
                   for p in range(2)]
            yTI = [spool.tile([128, 128], F16, name=f"yTI_{p}", tag=f"yTI_{p}") for p in range(2)]
            for p in range(2):
                nc.gpsimd.memset(yTE[p][:], 0.0)
                nc.gpsimd.memset(yTI[p][:], 0.0)

            # ext stream buffers (only partitions 0:16 carry data)
            NEXT = 4
            ext_t = [epool.tile([128, NE], F16, name=f"ext_sb{i}", tag=f"ext{i}") for i in range(NEXT)]
            for i in range(NEXT):
                nc.gpsimd.memset(ext_t[i][:], 0.0)

            # staging (relu(0.1*pre), fp16) and psum
            staging = stpool.tile([128, 1280], F16, name="staging_sb", tag="staging")
            ps_slot = [prepool.tile([128, 512], F32, name=f"psum_s{u}", tag=f"pre{u}")
                       for u in range(3)]
            for u in range(3):
                nc.vector.memset(ps_slot[u][:], 0.0)

            def stat_ap(src, k, par):
                if src in I_SLOT:
                    off = (4 * k + I_SLOT[src]) * 16
                    return yTI[par][:, off:off + 16]
                off = 128 * EORDER.index(src) + 16 * k
                return yTE[par][:, off:off + 16]

            def group_mm_list(g, t):
                """Ordered (lhsT_kind, rhs_ap_args, slot, first, last) for col
                group g's full step: slot0 E-unit, slot1 E-unit (+ext for S),
                slot2 I-unit."""
                out = []
                for slot in (0, 1):
                    dst, n0 = SLOT_E[(slot, g)]
                    contr = CONTR_E[dst]
                    n = len(contr) + (1 if dst == "S" else 0)
                    for j, (src, k) in enumerate(contr):
                        out.append((("y", src, k), (dst, j * NE + n0, 512),
                                    slot, j == 0, j == n - 1))
                    if dst == "S":
                        out.append((("ext", t, n0), None, slot, False, True))
                contr_i = CONTR_I[GROUPS[g]]
                for j, (src, k) in enumerate(contr_i):
                    out.append((("y", src, k), (GROUPS[g] + "i", j * NI, NI),
                                2, j == 0, j == len(contr_i) - 1))
                return out

            SLOT_OFF = {0: 0, 1: 512, 2: 1024}
            SLOT_W = {0: 512, 1: 512, 2: 256}

            def emit_step_mms(t, par):
                lists = [group_mm_list(g, t) for g in range(4)]
                for j in range(max(len(l) for l in lists)):
                    for g in range(4):
                        if j >= len(lists[g]):
                            continue
                        stat_kind, rhs_args, slot, first, last = lists[g][j]
                        outp = ps_slot[slot][32 * g:32 * g + 16, 0:SLOT_W[slot]]
                        if stat_kind[0] == "ext":
                            _, tt, n0 = stat_kind
                            lhsT = eye_e[:]
                            rhs = ext_t[tt % NEXT][:, n0:n0 + 512]
                        else:
                            _, src, k = stat_kind
                            lhsT = stat_ap(src, k, par)
                            wname, c0, w = rhs_args
                            rhs = w_sb[wname][:, c0:c0 + w]
                        nc.tensor.matmul(outp, lhsT, rhs, start=first, stop=last,
                                         tile_position=(0, 32 * g))

            def window(w, par, nxt):
                """Transpose staging window w (128 cols) and y' = 0.9y + r."""
                trt = trpool.tile([128, 128], F16, name="tr_ps", tag="tr")
                nc.tensor.transpose(trt[:], staging[:, 128 * w:128 * w + 128],
                                    ident[:])
                trv = trt[:].rearrange("p (g x) -> p g x", g=4)[:, :, 0:16]
                if w >= 8:  # I windows: chunk h of all four I states
                    h = w - 8
                    newv = yTI[nxt][:, 64 * h:64 * h + 64].rearrange(
                        "p (g x) -> p g x", g=4)
                    oldv = yTI[par][:, 64 * h:64 * h + 64].rearrange(
                        "p (g x) -> p g x", g=4)
                else:  # E windows: slot = w//4, h = w%4
                    slot, h = divmod(w, 4)
                    base = 256 * slot
                    newv = yTE[nxt][:, base:base + 256].rearrange(
                        "p (c x) -> p c x", c=4)[:, :, 16 * h:16 * h + 16]
                    oldv = yTE[par][:, base:base + 256].rearrange(
                        "p (c x) -> p c x", c=4)[:, :, 16 * h:16 * h + 16]
                nc.vector.tensor_scalar_mul(newv, oldv, 1.0 - DT)
                nc.vector.tensor_add(newv, newv, trv)

            for t in range(t_steps):
                par, nxt = t % 2, (t + 1) % 2
                nc.sync.dma_start(ext_t[t % NEXT][0:16, :],
                                  d_ext[16 * t:16 * t + 16, :])
                emit_step_mms(t, par)
                for slot in (2, 0, 1):
                    off, wd = SLOT_OFF[slot], SLOT_W[slot]
                    nc.scalar.activation(
                        staging[:, off:off + wd], ps_slot[slot][:, 0:wd],
                        mybir.ActivationFunctionType.Relu, scale=DT)
                    ws = [8, 9] if slot == 2 else [4 * slot, 4 * slot + 1,
                                                   4 * slot + 2, 4 * slot + 3]
                    for w in ws:
                        window(w, par, nxt)
                nc.sync.dma_start(d_gt[t], yTE[nxt][:, 0:128])

        # ================= Phase C: projection =================
        with (
            tc.tile_pool(name="wout", bufs=1) as wopool,
            tc.tile_pool(name="gt", bufs=1) as gtpool,
            tc.tile_pool(name="ostage", bufs=4) as opool,
            tc.tile_pool(name="psO", bufs=4, space="PSUM") as pspool,
        ):
            wout = wopool.tile([128, KT_E * VSH], F16, name="wout_sb", tag="wout")
            nc.sync.dma_start(wout[:], d_wout[:])
            gt_sb = []
            for c in range(KT_E):
                gtile = gtpool.tile([128, T_FULL * 16], F16, name=f"gt_sb{c}", tag=f"gt{c}")
                src = d_gt[:, :, 16 * c:16 * c + 16].rearrange("t p b -> p t b")
                nc.sync.dma_start(gtile[:].rearrange("p (t b) -> p t b", t=T_FULL),
                                  src)
                gt_sb.append(gtile)
            n_mt = (t_steps * 16) // 128
            for m in range(n_mt):
                for n in range(VSH // NCH):
                    ps = pspool.tile([128, NCH], F32, name="ps_o", tag="psO")
                    for k in range(KT_E):
                        nc.tensor.matmul(
                            ps[:], gt_sb[k][:, 128 * m:128 * (m + 1)],
                            wout[:, k * VSH + n * NCH:k * VSH + (n + 1) * NCH],
                            start=(k == 0), stop=(k == KT_E - 1))
                    ost = opool.tile([128, NCH], F32, name="ostage_sb", tag="ostage")
                    nc.vector.tensor_copy(ost[:], ps[:])
                    nc.sync.dma_start(
                        d_log[128 * m:128 * (m + 1), n * NCH:(n + 1) * NCH],
                        ost[:])

    nc.compile()
    return nc


def kernel(**inputs) -> np.ndarray:
    common, per_core = host_prepare(inputs)
    nc = build_program(T_FULL)
    in_maps = [dict(common, **per_core[c]) for c in range(NCORES)]
    res = run_bass_kernel_spmd(nc, in_maps, core_ids=list(range(NCORES)))
    parts = [res.results[c]["logits"] for c in range(NCORES)]
    full = np.concatenate(parts, axis=1)              # [T*B, V]
    out = full.reshape(T_FULL, B, V).transpose(1, 0, 2)  # (B, T, V)
    return np.ascontiguousarray(out.astype(np.float32))


# revision 11
# speedup vs baseline: 1.7981x; 1.7981x over previous
"""Trainium2 Bass kernel for the BIGA E/I recurrent network (gnn_message_passing).

Architecture recap (hardcoded shapes):
  tokens (16,128) -> x = emb[tokens]+pe -> ext = x @ W_in.T  (host, fp32)
  128 sequential Euler steps over 4 coupled E/I groups (S, A1, A2, G),
  per-step readout logits_t = ge @ W_out.T with V=32000.

Device strategy (8 NeuronCores, SPMD):
  - The recurrence is replicated on every core (B=16 gives no cross-core
    scaling: matmul cost is weight-streaming-bound, and per-step collectives
    have a ~20us floor).  All recurrence matmuls run in fp16 with the batch
    (M=16) as the stationary free dim and 4-way PE column tiling
    (tile_position=(0,32g)) so four weight streams run concurrently.
  - States are kept TRANSPOSED (yT[k,b], fp16) so they can be used directly
    as matmul stationaries; each step the new pre-activations are
    relu(0.1*x)-evacuated by ScalarE, transposed back in batched 128x128 PE
    transposes, and combined y' = 0.9*y + r on VectorE.
  - The V=32000 output projection is sharded over vocab: each core projects
    all T*B=2048 ge states onto its 4000-row slice of W_out with full
    128x128 matmuls, then the host concatenates.
"""

import os
import sys

import numpy as np

try:
    import concourse.bass as bass  # noqa: F401
except ImportError:
    for p in ("/opt/trn_rl_repo", "/root/.axon_site/_ro/trn_rl_repo"):
        if os.path.isdir(p) and p not in sys.path:
            sys.path.insert(0, p)

import concourse.bass as bass
import concourse.mybir as mybir
import concourse.tile as tile
from concourse import bacc
from concourse.bass_utils import run_bass_kernel_spmd

F16 = mybir.dt.float16
F32 = mybir.dt.float32

V, D, NE, NI, B, T_FULL = 32000, 512, 1024, 256, 16, 128
DT = 0.1
NCORES = 8
VSH = V // NCORES          # 4000 vocab rows per core
NCH = 500                  # projection free-dim chunk (4000 = 8*500)
KT_E = NE // 128           # 8 k-tiles per E state
KT_I = NI // 128           # 2 k-tiles per I state

# state name -> (is_E, index)
E_STATES = ["se", "a1e", "a2e", "ge"]
I_STATES = ["si", "a1i", "a2i", "gi"]
I_SLOT = {"si": 0, "a1i": 1, "a2i": 2, "gi": 3}

# Per-destination contraction orders.  Host weight packing MUST match.
#   entry = (src_state, k_tile)
def _ks(s, n, o=0):
    return [(s, k + o) for k in range(n)]

CONTR_E = {
    "S":  _ks("se", 8) + _ks("si", 2),                    # + ext via eyeE
    "A1": _ks("a1e", 8) + _ks("a1i", 2) + _ks("se", 4) + _ks("a2i", 2),
    "A2": _ks("a2e", 8) + _ks("a2i", 2) + _ks("se", 4, 4) + _ks("a1i", 2),
    "G":  _ks("ge", 8) + _ks("gi", 2) + _ks("a1e", 8) + _ks("a2e", 8),
}
CONTR_I = {
    "S":  _ks("se", 8) + _ks("si", 2),
    "A1": _ks("a1e", 8) + _ks("a1i", 2),
    "A2": _ks("a2e", 8) + _ks("a2i", 2),
    "G":  _ks("ge", 8) + _ks("gi", 2),
}
GROUPS = ["S", "A1", "A2", "G"]
E_OF = {"S": "se", "A1": "a1e", "A2": "a2e", "G": "ge"}
I_OF = {"S": "si", "A1": "a1i", "A2": "a2i", "G": "gi"}

# Column-group work assignment (PE 128x32 col tiling).  Each col group g
# owns 3 PSUM banks (slots); slot u of group g accumulates one unit:
#   slot 0 (free [0:512)):    g0: G[0:512)   g1: G[512:1024)  g2: A1[0:512) g3: A1[512:1024)
#   slot 1 (free [512:1024)): g0: S[0:512)   g1: S[512:1024)  g2: A2[0:512) g3: A2[512:1024)
#   slot 2 (free [1024:1280)): I unit of group g's own state (N=256)
# Per-group cycles: g0/g1: 26*512+11*512+10*256 = 21504; g2/g3: 18944.
SLOT_E = {  # (slot, g) -> (dst_group, n0)
    (0, 0): ("G", 0), (0, 1): ("G", 512), (0, 2): ("A1", 0), (0, 3): ("A1", 512),
    (1, 0): ("S", 0), (1, 1): ("S", 512), (1, 2): ("A2", 0), (1, 3): ("A2", 512),
}
PRE_W = 1536  # psum_pre / staging free width (3 banks per partition-group)


def _pack_kxn(blocks):
    """blocks: list of [128, N] fp32 arrays (the k-tile slices of W.T, in
    contraction order).  Returns [128, nk*N] fp16 (k-tile-major columns)."""
    a = np.stack(blocks, axis=0)            # [nk, 128, N]
    a = np.transpose(a, (1, 0, 2))          # [128, nk, N]
    return np.ascontiguousarray(a.reshape(128, -1)).astype(np.float16)


def _kslices(wT, nk):
    """wT: [K, N] with K = nk*128 -> list of nk [128, N] blocks."""
    return [wT[128 * k:128 * (k + 1)] for k in range(nk)]


def host_prepare(inputs):
    """All constant/layout prep in numpy. Returns (common_map, per_core_maps)."""
    f = {k: np.asarray(v, dtype=np.float32) if np.asarray(v).dtype != np.int64
         and np.asarray(v).dtype != np.int32 else np.asarray(v)
         for k, v in inputs.items()}
    tokens = np.asarray(inputs["tokens"]).astype(np.int64)

    # ext = (emb[tokens] + pe) @ W_in.T  -> (T, B, NE) -> [T*B, NE] fp16
    x = f["embedding"][tokens] + f["pos_enc"][None, :T_FULL]      # (B,T,D)
    ext = np.einsum("btd,nd->btn", x.astype(np.float32),
                    f["W_in"].astype(np.float32))                  # (B,T,NE)
    ext = np.transpose(ext, (1, 0, 2)).reshape(T_FULL * B, NE)     # t-major
    ext16 = ext.astype(np.float16)

    wmap = {}
    # E-destination packs (N=1024): order must match CONTR_E
    wmap["w_S"] = _pack_kxn(_kslices(f["S_Wee"].T, 8) + _kslices(-f["S_Wei"].T, 2))
    wmap["w_A1"] = _pack_kxn(
        _kslices(f["A1_Wee"].T, 8) + _kslices(-f["A1_Wei"].T, 2)
        + _kslices(f["C_S_A1"].T[:512], 4) + _kslices(-f["L_A2_A1"].T, 2))
    wmap["w_A2"] = _pack_kxn(
        _kslices(f["A2_Wee"].T, 8) + _kslices(-f["A2_Wei"].T, 2)
        + _kslices(f["C_S_A2"].T[512:], 4) + _kslices(-f["L_A1_A2"].T, 2))
    wmap["w_G"] = _pack_kxn(
        _kslices(f["G_Wee"].T, 8) + _kslices(-f["G_Wei"].T, 2)
        + _kslices(f["C_A1_G"].T, 8) + _kslices(f["C_A2_G"].T, 8))
    # I-destination packs (N=256)
    for gname in GROUPS:
        wmap[f"w_{gname}i"] = _pack_kxn(
            _kslices(f[f"{gname}_Wie"].T, 8) + _kslices(-f[f"{gname}_Wii"].T, 2))

    wmap["eye_e"] = np.eye(128, 16, dtype=np.float16)
    wmap["ident"] = np.eye(128, 128, dtype=np.float16)

    common = dict(ext=ext16, **wmap)

    # per-core W_out vocab shard, packed k-tile-major: [128, 8*4000]
    per_core = []
    for c in range(NCORES):
        wsh = f["W_out"][c * VSH:(c + 1) * VSH]        # [4000, NE]
        wT = wsh.T                                     # [NE, 4000]
        per_core.append({"w_out": _pack_kxn(_kslices(wT, 8))})
    return common, per_core


def build_program(t_steps=T_FULL, reps=1):
    nc = bacc.Bacc("TRN2", target_bir_lowering=False, debug=False,
                   num_devices=NCORES)

    # ---- DRAM I/O ----
    d_ext = nc.dram_tensor("ext", [T_FULL * B, NE], F16, kind="ExternalInput")
    d_w = {}
    for gname in GROUPS:
        d_w[gname] = nc.dram_tensor(f"w_{gname}", [128, len(CONTR_E[gname]) * NE],
                                    F16, kind="ExternalInput")
        d_w[gname + "i"] = nc.dram_tensor(f"w_{gname}i", [128, len(CONTR_I[gname]) * NI],
                                          F16, kind="ExternalInput")
    d_eye = nc.dram_tensor("eye_e", [128, 16], F16, kind="ExternalInput")
    d_ident = nc.dram_tensor("ident", [128, 128], F16, kind="ExternalInput")
    d_wout = nc.dram_tensor("w_out", [128, KT_E * VSH], F16, kind="ExternalInput")
    d_gt = nc.dram_tensor("gt_buf", [T_FULL, 128, 128], F16)
    d_log = nc.dram_tensor("logits", [T_FULL * B, VSH], F32, kind="ExternalOutput")

    from contextlib import ExitStack
    with tile.TileContext(nc) as tc, ExitStack() as _rep_ctx:
        if reps > 1:
            # timing-only variant: repeat the whole kernel body on-device so
            # wall-clock deltas isolate pure device execution time
            _rep_ctx.enter_context(tc.For_i(0, reps, 1))
        # ================= Phase B: recurrence =================
        with (
            tc.tile_pool(name="wpool", bufs=1) as wpool,
            tc.tile_pool(name="state", bufs=1) as spool,
            tc.tile_pool(name="ext", bufs=1) as epool,
            tc.tile_pool(name="stag", bufs=1) as stpool,
            tc.tile_pool(name="pre", bufs=1, space="PSUM") as prepool,
            tc.tile_pool(name="tr", bufs=3, space="PSUM") as trpool,
        ):
            # weights
            w_sb = {}
            for gname in GROUPS:
                w_sb[gname] = wpool.tile([128, len(CONTR_E[gname]) * NE], F16,
                                         name=f"w_{gname}", tag=f"w{gname}")
                nc.sync.dma_start(w_sb[gname][:], d_w[gname][:])
                w_sb[gname + "i"] = wpool.tile([128, len(CONTR_I[gname]) * NI],
                                               F16, name=f"w_{gname}i", tag=f"w{gname}i")
                nc.sync.dma_start(w_sb[gname + "i"][:], d_w[gname + "i"][:])
            eye_e = wpool.tile([128, 16], F16, name="eye_e_sb", tag="eye")
            nc.sync.dma_start(eye_e[:], d_eye[:])
            ident = wpool.tile([128, 128], F16, name="ident_sb", tag="ident")
            nc.sync.dma_start(ident[:], d_ident[:])

            # transposed states, double-buffered by step parity.
            # E states live in ONE [128, 512] tile per parity, state order
            # EORDER (chosen so each transpose window's 4 psum col-blocks map
            # to one strided AP).  I states: [128, 128], offset (4c+s)*16.
            EORDER = ["ge", "a1e", "se", "a2e"]
            yTE = [spool.tile([128, 512], F16, name=f"yTE_{p}", tag=f"yTE_{p}")
                   for p in range(2)]
            yTI = [spool.tile([128, 128], F16, name=f"yTI_{p}", tag=f"yTI_{p}") for p in range(2)]
            for p in range(2):
                nc.gpsimd.memset(yTE[p][:], 0.0)
                nc.gpsimd.memset(yTI[p][:], 0.0)

            # ext stream buffers (only partitions 0:16 carry data)
            NEXT = 4
            ext_t = [epool.tile([128, NE], F16, name=f"ext_sb{i}", tag=f"ext{i}") for i in range(NEXT)]
            for i in range(NEXT):
                nc.gpsimd.memset(ext_t[i][:], 0.0)

            # staging (relu(0.1*pre), fp16) and psum
            staging = stpool.tile([128, 1280], F16, name="staging_sb", tag="staging")
            ps_slot = [prepool.tile([128, 512], F32, name=f"psum_s{u}", tag=f"pre{u}")
                       for u in range(3)]
            for u in range(3):
                nc.vector.memset(ps_slot[u][:], 0.0)

            def stat_ap(src, k, par):
                if src in I_SLOT:
                    off = (4 * k + I_SLOT[src]) * 16
                    return yTI[par][:, off:off + 16]
                off = 128 * EORDER.index(src) + 16 * k
                return yTE[par][:, off:off + 16]

            def group_mm_list(g, t):
                """Ordered (lhsT_kind, rhs_ap_args, slot, first, last) for col
                group g's full step: slot0 E-unit, slot1 E-unit (+ext for S),
                slot2 I-unit."""
                out = []
                for slot in (0, 1):
                    dst, n0 = SLOT_E[(slot, g)]
                    contr = CONTR_E[dst]
                    n = len(contr) + (1 if dst == "S" else 0)
                    for j, (src, k) in enumerate(contr):
                        out.append((("y", src, k), (dst, j * NE + n0, 512),
                                    slot, j == 0, j == n - 1))
                    if dst == "S":
                        out.append((("ext", t, n0), None, slot, False, True))
                contr_i = CONTR_I[GROUPS[g]]
                for j, (src, k) in enumerate(contr_i):
                    out.append((("y", src, k), (GROUPS[g] + "i", j * NI, NI),
                                2, j == 0, j == len(contr_i) - 1))
                return out

            SLOT_OFF = {0: 0, 1: 512, 2: 1024}
            SLOT_W = {0: 512, 1: 512, 2: 256}

            def emit_step_mms(t, par):
                lists = [group_mm_list(g, t) for g in range(4)]
                for j in range(max(len(l) for l in lists)):
                    for g in range(4):
                        if j >= len(lists[g]):
                            continue
                        stat_kind, rhs_args, slot, first, last = lists[g][j]
                        outp = ps_slot[slot][32 * g:32 * g + 16, 0:SLOT_W[slot]]
                        if stat_kind[0] == "ext":
                            _, tt, n0 = stat_kind
                            lhsT = eye_e[:]
                            rhs = ext_t[tt % NEXT][:, n0:n0 + 512]
                        else:
                            _, src, k = stat_kind
                            lhsT = stat_ap(src, k, par)
                            wname, c0, w = rhs_args
                            rhs = w_sb[wname][:, c0:c0 + w]
                        # skip_group_check: CoreSim's zero-region conflict
                        # check flat-aliases partition offsets, falsely
                        # flagging concurrent groups at DISJOINT partitions
                        # (its value semantics, and HW has_written, are
                        # per-partition and handle this correctly).
                        nc.tensor.matmul(outp, lhsT, rhs, start=first, stop=last,
                                         tile_position=(0, 32 * g),
                                         skip_group_check=True)

            def window(w, par, nxt):
                """Transpose staging window w (128 cols) and y' = 0.9y + r."""
                trt = trpool.tile([128, 128], F16, name="tr_ps", tag="tr")
                nc.tensor.transpose(trt[:], staging[:, 128 * w:128 * w + 128],
                                    ident[:])
                trv = trt[:].rearrange("p (g x) -> p g x", g=4)[:, :, 0:16]
                if w >= 8:  # I windows: chunk h of all four I states
                    h = w - 8
                    newv = yTI[nxt][:, 64 * h:64 * h + 64].rearrange(
                        "p (g x) -> p g x", g=4)
                    oldv = yTI[par][:, 64 * h:64 * h + 64].rearrange(
                        "p (g x) -> p g x", g=4)
                else:  # E windows: slot = w//4, h = w%4
                    slot, h = divmod(w, 4)
                    base = 256 * slot
                    newv = yTE[nxt][:, base:base + 256].rearrange(
                        "p (c x) -> p c x", c=4)[:, :, 16 * h:16 * h + 16]
                    oldv = yTE[par][:, base:base + 256].rearrange(
                        "p (c x) -> p c x", c=4)[:, :, 16 * h:16 * h + 16]
                nc.vector.tensor_scalar_mul(newv, oldv, 1.0 - DT)
                nc.vector.tensor_add(newv, newv, trv)

            for t in range(t_steps):
                par, nxt = t % 2, (t + 1) % 2
                nc.sync.dma_start(ext_t[t % NEXT][0:16, :],
                                  d_ext[16 * t:16 * t + 16, :])
                emit_step_mms(t, par)
                for slot in (2, 0, 1):
                    off, wd = SLOT_OFF[slot], SLOT_W[slot]
                    nc.scalar.activation(
                        staging[:, off:off + wd], ps_slot[slot][:, 0:wd],
                        mybir.ActivationFunctionType.Relu, scale=DT)
                    ws = [8, 9] if slot == 2 else [4 * slot, 4 * slot + 1,
                                                   4 * slot + 2, 4 * slot + 3]
                    for w in ws:
                        window(w, par, nxt)
                nc.sync.dma_start(d_gt[t], yTE[nxt][:, 0:128])

        # ================= Phase C: projection =================
        with (
            tc.tile_pool(name="wout", bufs=1) as wopool,
            tc.tile_pool(name="gt", bufs=1) as gtpool,
            tc.tile_pool(name="ostage", bufs=4) as opool,
            tc.tile_pool(name="psO", bufs=4, space="PSUM") as pspool,
        ):
            wout = wopool.tile([128, KT_E * VSH], F16, name="wout_sb", tag="wout")
            nc.sync.dma_start(wout[:], d_wout[:])
            gt_sb = []
            for c in range(KT_E):
                gtile = gtpool.tile([128, T_FULL * 16], F16, name=f"gt_sb{c}", tag=f"gt{c}")
                src = d_gt[:, :, 16 * c:16 * c + 16].rearrange("t p b -> p t b")
                nc.sync.dma_start(gtile[:].rearrange("p (t b) -> p t b", t=T_FULL),
                                  src)
                gt_sb.append(gtile)
            n_mt = (t_steps * 16) // 128
            for m in range(n_mt):
                for n in range(VSH // NCH):
                    ps = pspool.tile([128, NCH], F32, name="ps_o", tag="psO")
                    for k in range(KT_E):
                        nc.tensor.matmul(
                            ps[:], gt_sb[k][:, 128 * m:128 * (m + 1)],
                            wout[:, k * VSH + n * NCH:k * VSH + (n + 1) * NCH],
                            start=(k == 0), stop=(k == KT_E - 1))
                    ost = opool.tile([128, NCH], F32, name="ostage_sb", tag="ostage")
                    nc.vector.tensor_copy(ost[:], ps[:])
                    nc.sync.dma_start(
                        d_log[128 * m:128 * (m + 1), n * NCH:(n + 1) * NCH],
                        ost[:])

    nc.compile()
    return nc


def kernel(**inputs) -> np.ndarray:
    common, per_core = host_prepare(inputs)
    nc = build_program(T_FULL)
    in_maps = [dict(common, **per_core[c]) for c in range(NCORES)]
    res = run_bass_kernel_spmd(nc, in_maps, core_ids=list(range(NCORES)))
    parts = [res.results[c]["logits"] for c in range(NCORES)]
    full = np.concatenate(parts, axis=1)              # [T*B, V]
    out = full.reshape(T_FULL, B, V).transpose(1, 0, 2)  # (B, T, V)
    return np.ascontiguousarray(out.astype(np.float32))
